# revision 15
# baseline (speedup 1.0000x reference)
"""Concept Whitening layer (IterNorm ZCA + rotation) as a Trainium2 Bass/Tile kernel.

Strategy (8-way data parallel over batch), v3 — bf16 compute, barrier-absorbed
AllReduce, fused Newton:
  - Each core holds 8 of the 64 batches.  x arrives fp32 (12.8 MB/core) and is
    cast to a bf16 SBUF-resident copy (DVE/ACT alternating, half-batch chunks).
    Output tolerance is 2e-2; full-bf16 numerics measure ~3e-3 end to end.
  - A 32-byte dummy AllReduce is issued first: the one-time collective-stream
    init barrier (measured 40-119us, gated on cross-core launch skew) completes
    during phase 1 instead of serializing in front of the real collective.
  - Phase 1: per 128-column chunk, PE-transpose (bf16) the chunk, evict to a
    rotating SBUF strip with a ones-column appended, accumulate
    [G | s] = y^T [y | 1] into one fp32 PSUM tile (196 accumulating matmuls).
  - AllReduce the (128,129) [G|s] (AG and AR both measure ~26us here; AR saves
    the 8 gather DMAs + 7-add local reduce tree).
  - Phase 2 (replicated): trace via diag mask + ones-matmul broadcast,
    rtr = 1/tr(Sigma), Newton for Sigma^{-1/2} in bf16 with P1 = 1.5I - 0.5*SigN
    computed analytically (saves one iteration) and the update fused into PSUM:
    evict P^2 scaled by -0.5 (exact in bf16), c = (-0.5 P^2)(P Sig), then
    c += P @ (1.5 I) accumulated in PSUM -> single evict per iteration.
  - Phase 3: out = M @ x - bias as 512-col bf16 matmuls over the resident
    x_bf16, PSUM->SBUF eviction with fused bias add alternating DVE/ACT,
    streamed to HBM per batch (double buffered).

out[b,d,h,w] = sum_c rot[d,c] * (wm @ (x-mean))[c] == (rot@wm) @ x - (rot@wm) @ mean.
"""

import sys

try:
    import concourse  # noqa: F401  (normally on PYTHONPATH in this container)
except ImportError:
    sys.path.insert(0, "/opt/trn_rl_repo")

from contextlib import ExitStack

import numpy as np

import concourse.bacc as bacc
import concourse.bass as bass
import concourse.mybir as mybir
import concourse.tile as tile
from concourse import bass_utils

# Problem constants (hardcoded per harness contract).
B, C, H, W = 64, 128, 56, 56
HW = H * W                    # 3136
M_TOT = B * HW                # 200704
N_CORES = 8
B_LOC = B // N_CORES          # 8
M_LOC = B_LOC * HW            # 25088
N_CHUNK = M_LOC // 128        # 196
T_NEWTON = 5
EPS = 1e-5

FP32 = mybir.dt.float32
BF16 = mybir.dt.bfloat16
AX = mybir.AxisListType
ALU = mybir.AluOpType
ACTF = mybir.ActivationFunctionType

NP_BF16 = mybir.dt.np(BF16)


def _build_program(b_loc=B_LOC):
    hw = HW
    m_loc = b_loc * hw
    n_chunk = m_loc // 128
    assert n_chunk * 128 == m_loc
    m_tot = N_CORES * m_loc
    nc = bacc.Bacc(
        "TRN2",
        target_bir_lowering=False,
        debug=False,
        enable_asserts=False,
        num_devices=N_CORES,
    )

    x_dram = nc.dram_tensor("x", [b_loc, C, hw], FP32, kind="ExternalInput")
    rot_dram = nc.dram_tensor("rot", [C, C], FP32, kind="ExternalInput")
    out_dram = nc.dram_tensor("out", [b_loc, C, hw], FP32, kind="ExternalOutput")

    with tile.TileContext(nc) as tc, ExitStack() as stack:
        consts = stack.enter_context(tc.tile_pool(name="consts", bufs=1))
        persist = stack.enter_context(tc.tile_pool(name="persist", bufs=1))

        # Constants via inline (NEFF-embedded) tensors.
        eye_bf_dram = nc.inline_tensor(np.eye(C).astype(NP_BF16), name="c_eye_bf")
        eye15_bf_dram = nc.inline_tensor(
            (1.5 * np.eye(C)).astype(NP_BF16), name="c_eye15_bf"
        )
        eye_f_dram = nc.inline_tensor(np.eye(C, dtype=np.float32), name="c_eye_f")
        ones_bf_dram = nc.inline_tensor(np.ones((C, C)).astype(NP_BF16), name="c_ones_bf")
        # --- dummy collective: absorbs the one-time cc-stream init barrier ---
        dummy_sb = consts.tile([1, 8], FP32)
        nc.vector.memset(dummy_sb, 0.0)
        with tc.tile_pool(name="dram_dummy", bufs=1, space="DRAM") as dummy_pool:
            dummy_in = dummy_pool.tile([1, 8], FP32)
            dummy_out = dummy_pool.tile([1, 8], FP32, addr_space="Shared")
            nc.sync.dma_start(dummy_in, dummy_sb)
            nc.gpsimd.collective_compute(
                "AllReduce",
                ALU.add,
                replica_groups=[list(range(N_CORES))],
                ins=[dummy_in.opt()],
                outs=[dummy_out.opt()],
            )

        # --- load x and rot ---
        # batch 0 is loaded first, in halves, so the cast->transpose pipeline
        # starts ~4us earlier than with consts at the queue head.
        xsb = persist.tile([C, b_loc, hw], FP32)
        nc.sync.dma_start(out=xsb[:, 0, 0 : hw // 2], in_=x_dram[0, :, 0 : hw // 2])
        nc.sync.dma_start(out=xsb[:, 0, hw // 2 : hw], in_=x_dram[0, :, hw // 2 : hw])
        eye_bf = consts.tile([C, C], BF16)
        nc.sync.dma_start(eye_bf, eye_bf_dram[:])
        rot_sb = persist.tile([C, C], FP32)
        nc.sync.dma_start(out=rot_sb, in_=rot_dram[:])
        eye15_bf = consts.tile([C, C], BF16)
        nc.sync.dma_start(eye15_bf, eye15_bf_dram[:])
        eye_f = consts.tile([C, C], FP32)
        nc.sync.dma_start(eye_f, eye_f_dram[:])
        ones_bf = consts.tile([C, C], BF16)
        nc.sync.dma_start(ones_bf, ones_bf_dram[:])
        for b in range(1, b_loc):
            nc.sync.dma_start(out=xsb[:, b, :], in_=x_dram[b])
        xflat = xsb.rearrange("p a b -> p (a b)")

        # bf16 copy of x, cast in half-batch chunks alternating DVE/ACT.
        xbf = persist.tile([C, m_loc], BF16)
        half = hw // 2  # 1568
        for k in range(2 * b_loc):
            dst = xbf[:, k * half : (k + 1) * half]
            src = xflat[:, k * half : (k + 1) * half]
            if k % 2 == 0:
                nc.vector.tensor_copy(dst, src)
            else:
                nc.scalar.copy(dst, src)

        # rot^T in bf16 (independent of stats; runs during phase 1)
        rot_bf = persist.tile([C, C], BF16)
        nc.vector.tensor_copy(rot_bf, rot_sb)

        # --- phase 1: Gram + channel sums, all bf16 on the PE ---
        # Transposed chunks are evicted PSUM->SBUF four-at-a-time (one strided
        # copy per 4 chunks): phase 1 was eviction-paced at ~290ns/chunk with
        # per-chunk copies; quad eviction brings it to ~100ns/chunk so the PE
        # (MMs issue ~56-81ns apart warm, ldweights hidden by the background
        # weight buffer) sets the pace instead.
        N_QSTRIP = 6
        qstrips = [
            persist.tile([C, 4, C + 1], BF16, name=f"qstrip{i}") for i in range(N_QSTRIP)
        ]
        for qs in qstrips:
            nc.vector.memset(qs[:, :, C : C + 1], 1.0)

        rotT_bf = persist.tile([C, C], BF16)

        with (
            tc.tile_pool(name="ph1_psum", bufs=6, space=bass.MemorySpace.PSUM) as ph1_psum,
            tc.tile_pool(name="gs_psum_pool", bufs=1, space=bass.MemorySpace.PSUM) as gs_pool,
        ):
            # rot transpose (PE, once)
            rotT_ps = gs_pool.tile([C, C], BF16, tag="rott")
            nc.tensor.transpose(rotT_ps, rot_bf, eye_bf)
            nc.scalar.copy(rotT_bf, rotT_ps)

            gs_psum = gs_pool.tile([C, C + 1], FP32)
            n_quad = n_chunk // 4  # 49
            for q in range(n_quad):
                y_ps = ph1_psum.tile([C, 4 * C], BF16, tag="ytrans")
                ypsv = y_ps.rearrange("p (s c) -> p s c", s=4)
                for s in range(4):
                    j = 4 * q + s
                    nc.tensor.transpose(
                        ypsv[:, s, :], xbf[:, j * 128 : (j + 1) * 128], eye_bf
                    )
                qs = qstrips[q % N_QSTRIP]
                if q % 2 == 0:
                    nc.vector.tensor_copy(qs[:, :, 0:C], ypsv)
                else:
                    nc.scalar.copy(qs[:, :, 0:C], ypsv)
                for s in range(4):
                    j = 4 * q + s
                    nc.tensor.matmul(
                        gs_psum,
                        qs[:, s, 0:C],
                        qs[:, s, 0 : C + 1],
                        start=(j == 0),
                        stop=(j == n_chunk - 1),
                    )

            gs_sb = persist.tile([C, C + 1], FP32)
            nc.vector.tensor_copy(gs_sb, gs_psum)

        # --- AllReduce [G|s] across the 8 cores ---
        gtot = persist.tile([C, C + 1], FP32)
        with tc.tile_pool(name="dram", bufs=1, space="DRAM") as dram_pool:
            cc_in = dram_pool.tile([C, C + 1], FP32)
            cc_out = dram_pool.tile([C, C + 1], FP32, addr_space="Shared")
            nc.sync.dma_start(cc_in, gs_sb)
            nc.gpsimd.collective_compute(
                "AllReduce",
                ALU.add,
                replica_groups=[list(range(N_CORES))],
                ins=[cc_in.opt()],
                outs=[cc_out.opt()],
            )
            nc.sync.dma_start(gtot, cc_out)

        # --- phase 2: small replicated math ---
        with tc.tile_pool(name="ph2_psum", bufs=4, space=bass.MemorySpace.PSUM) as pp:
            inv_m = float(1.0 / m_tot)
            mean_bf = persist.tile([C, 1], BF16)
            nc.vector.tensor_scalar_mul(mean_bf, gtot[:, C : C + 1], inv_m)

            # trace: diag extract (single nonzero per row -> exact even in bf16)
            dmul_bf = persist.tile([C, C], BF16)
            nc.vector.tensor_mul(dmul_bf, gtot[:, 0:C], eye_f)
            diag_bf = persist.tile([C, 1], BF16)
            with nc.allow_low_precision(reason="single nonzero per row; exact"):
                nc.vector.tensor_reduce(diag_bf, dmul_bf, AX.X, ALU.add)
            trace_ps = pp.tile([C, 1], FP32, tag="ph2")
            nc.tensor.matmul(trace_ps, ones_bf, diag_bf, start=True, stop=True)
            # rtr = 1 / (trace(G)/m + C*eps)
            tr_sc = persist.tile([C, 1], FP32)
            nc.vector.tensor_scalar(
                tr_sc, trace_ps, inv_m, float(C * EPS), ALU.mult, ALU.add
            )
            rtr = persist.tile([C, 1], FP32)
            nc.vector.reciprocal(rtr, tr_sc)

            # sigN = (G/m) * rtr in bf16 (eps*rtr ~6e-8 vs diag ~8e-3: dropped)
            rtr_m = persist.tile([C, 1], FP32)
            nc.vector.tensor_scalar_mul(rtr_m, rtr, inv_m)
            sigN_bf = persist.tile([C, C], BF16)
            nc.vector.tensor_scalar_mul(sigN_bf, gtot[:, 0:C], rtr_m)

            # Newton in bf16: P <- 1.5 P - 0.5 P^3 SigmaN, P1 analytic.
            # P1 = 1.5 I - 0.5 SigmaN  (one DVE op + add of 1.5I const)
            p1_bf = persist.tile([C, C], BF16)
            nh_bf = persist.tile([C, C], BF16)
            nc.vector.tensor_scalar_mul(nh_bf, sigN_bf, -0.5)
            nc.vector.tensor_add(p1_bf, nh_bf, eye15_bf)
            pcur = p1_bf
            # remaining iterations with the update fused into PSUM accumulation:
            #   a = -0.5 P^2 (scaled on evict; exact in bf16)
            #   c = a @ (P Sig) ; c += P @ (1.5 I)  -> c = 1.5P - 0.5 P^3 Sig
            ptiles = [persist.tile([C, C], BF16, name=f"pnewt{i}") for i in range(2)]
            ab_t = [persist.tile([C, C], BF16, name=f"abuf{i}") for i in range(2)]
            db_t = [persist.tile([C, C], BF16, name=f"dbuf{i}") for i in range(2)]
            for it in range(T_NEWTON - 1):
                a_bf, d_bf = ab_t[it % 2], db_t[it % 2]
                a_ps = pp.tile([C, C], FP32, tag="ph2")
                d_ps = pp.tile([C, C], FP32, tag="ph2")
                # d first: its eviction runs on the slower ACT engine
                nc.tensor.matmul(d_ps, pcur, sigN_bf, start=True, stop=True)  # P Sig
                nc.tensor.matmul(a_ps, pcur, pcur, start=True, stop=True)     # P^2
                nc.scalar.copy(d_bf, d_ps)
                nc.vector.tensor_scalar_mul(a_bf, a_ps, -0.5)
                c_ps = pp.tile([C, C], FP32, tag="ph2")
                nc.tensor.matmul(c_ps, a_bf, d_bf, start=True, stop=False)  # -0.5 P^3 S
                nc.tensor.matmul(c_ps, pcur, eye15_bf, start=False, stop=True)  # +1.5P
                pnext = ptiles[it % 2]
                nc.vector.tensor_copy(pnext, c_ps)
                pcur = pnext

            # srtr = sqrt(rtr) via 2 Newton steps on DVE, seed s0 = sqrt(1/128)
            # (emitted after the iteration loop so the DVE runs it during the
            # PE-bound stretches instead of in front of the critical chain)
            s0 = float(np.sqrt(1.0 / C))
            t_a = persist.tile([C, 1], FP32)
            nc.vector.tensor_scalar(
                t_a, rtr, 0.5 / s0, 0.5 * s0, ALU.mult, ALU.add
            )  # s1 = (rtr/s0 + s0)/2
            t_r = persist.tile([C, 1], FP32)
            nc.vector.reciprocal(t_r, t_a)                    # 1/s1
            t_b = persist.tile([C, 1], FP32)
            nc.vector.tensor_mul(t_b, rtr, t_r)               # rtr/s1
            srtr = persist.tile([C, 1], FP32)
            nc.vector.tensor_add(srtr, t_a, t_b)
            nc.vector.tensor_scalar_mul(srtr, srtr, 0.5)      # s2

            # MT = sqrt(rTr) * P rot^T = M^T  (P symmetric)
            mt_ps = pp.tile([C, C], FP32, tag="ph2")
            nc.tensor.matmul(mt_ps, pcur, rotT_bf, start=True, stop=True)
            mt_bf = persist.tile([C, C], BF16)
            nc.vector.tensor_scalar_mul(mt_bf, mt_ps, srtr)

            # negbias = -(M @ mean)
            nb_ps = pp.tile([C, 1], FP32, tag="ph2")
            nc.tensor.matmul(nb_ps, mt_bf, mean_bf, start=True, stop=True)
            nb_sb = persist.tile([C, 1], FP32)
            nc.vector.tensor_scalar_mul(nb_sb, nb_ps, -1.0)

        # --- phase 3: out = M @ x - bias ---
        n_full, rem = divmod(hw, 512)  # 6, 64
        widths = [512] * n_full + ([rem] if rem else [])
        with (
            tc.tile_pool(name="ph3_psum", bufs=4, space=bass.MemorySpace.PSUM) as op_ps,
            tc.tile_pool(name="outsb_pool", bufs=2) as outsb_pool,
        ):
            for b in range(b_loc):
                osb = outsb_pool.tile([C, hw], FP32)
                col = 0
                for k, wdt in enumerate(widths):
                    ops = op_ps.tile([C, 512], FP32, tag="ops")
                    nc.tensor.matmul(
                        ops[:, 0:wdt],
                        mt_bf,
                        xbf[:, b * hw + col : b * hw + col + wdt],
                        start=True,
                        stop=True,
                    )
                    if k % 2 == 0:
                        nc.vector.tensor_scalar_add(
                            osb[:, col : col + wdt], ops[:, 0:wdt], nb_sb
                        )
                    else:
                        nc.scalar.add(
                            osb[:, col : col + wdt], ops[:, 0:wdt], nb_sb[:, 0:1]
                        )
                    col += wdt
                # split the writeback so streaming starts mid-batch
                nc.sync.dma_start(out=out_dram[b, :, 0:1536], in_=osb[:, 0:1536])
                nc.sync.dma_start(out=out_dram[b, :, 1536:hw], in_=osb[:, 1536:hw])

    nc.compile()
    return nc


_PROGRAM = None


def _get_program():
    global _PROGRAM
    if _PROGRAM is None:
        _PROGRAM = _build_program()
    return _PROGRAM


LAST_RESULTS = None


def kernel(x: np.ndarray, running_rot: np.ndarray) -> np.ndarray:
    global LAST_RESULTS
    x = np.ascontiguousarray(np.asarray(x, dtype=np.float32))
    rot = np.ascontiguousarray(np.asarray(running_rot, dtype=np.float32))
    assert x.shape == (B, C, H, W) and rot.shape == (C, C)

    nc = _get_program()
    xr = x.reshape(N_CORES, B_LOC, C, HW)
    in_maps = [{"x": xr[i], "rot": rot} for i in range(N_CORES)]
    res = bass_utils.run_bass_kernel_spmd(nc, in_maps, list(range(N_CORES)))
    LAST_RESULTS = res

    out = np.empty((B, C, H, W), dtype=np.float32)
    for i in range(N_CORES):
        out[i * B_LOC : (i + 1) * B_LOC] = res.results[i]["out"].reshape(
            B_LOC, C, H, W
        )
    return out


# revision 19
# speedup vs baseline: 1.0103x; 1.0103x over previous
"""Concept Whitening layer (IterNorm ZCA + rotation) as a Trainium2 Bass/Tile kernel.

Strategy (8-way data parallel over batch), v3 — bf16 compute, barrier-absorbed
AllReduce, fused Newton:
  - Each core holds 8 of the 64 batches.  x arrives fp32 (12.8 MB/core) and is
    cast to a bf16 SBUF-resident copy (DVE/ACT alternating, half-batch chunks).
    Output tolerance is 2e-2; full-bf16 numerics measure ~3e-3 end to end.
  - A 32-byte dummy AllReduce is issued first: the one-time collective-stream
    init barrier (measured 40-119us, gated on cross-core launch skew) completes
    during phase 1 instead of serializing in front of the real collective.
  - Phase 1: per 128-column chunk, PE-transpose (bf16) the chunk, evict to a
    rotating SBUF strip with a ones-column appended, accumulate
    [G | s] = y^T [y | 1] into one fp32 PSUM tile (196 accumulating matmuls).
  - AllReduce the (128,129) [G|s] (AG and AR both measure ~26us here; AR saves
    the 8 gather DMAs + 7-add local reduce tree).
  - Phase 2 (replicated): trace via diag mask + ones-matmul broadcast,
    rtr = 1/tr(Sigma), Newton for Sigma^{-1/2} in bf16 with P1 = 1.5I - 0.5*SigN
    computed analytically (saves one iteration) and the update fused into PSUM:
    evict P^2 scaled by -0.5 (exact in bf16), c = (-0.5 P^2)(P Sig), then
    c += P @ (1.5 I) accumulated in PSUM -> single evict per iteration.
  - Phase 3: out = M @ x - bias as 512-col bf16 matmuls over the resident
    x_bf16, PSUM->SBUF eviction with fused bias add alternating DVE/ACT,
    streamed to HBM per batch (double buffered).

out[b,d,h,w] = sum_c rot[d,c] * (wm @ (x-mean))[c] == (rot@wm) @ x - (rot@wm) @ mean.
"""

import sys

try:
    import concourse  # noqa: F401  (normally on PYTHONPATH in this container)
except ImportError:
    sys.path.insert(0, "/opt/trn_rl_repo")

from contextlib import ExitStack

import numpy as np

import concourse.bacc as bacc
import concourse.bass as bass
import concourse.mybir as mybir
import concourse.tile as tile
from concourse import bass_utils

# Problem constants (hardcoded per harness contract).
B, C, H, W = 64, 128, 56, 56
HW = H * W                    # 3136
M_TOT = B * HW                # 200704
N_CORES = 8
B_LOC = B // N_CORES          # 8
M_LOC = B_LOC * HW            # 25088
N_CHUNK = M_LOC // 128        # 196
T_NEWTON = 5
EPS = 1e-5

FP32 = mybir.dt.float32
BF16 = mybir.dt.bfloat16
AX = mybir.AxisListType
ALU = mybir.AluOpType
ACTF = mybir.ActivationFunctionType

NP_BF16 = mybir.dt.np(BF16)


def _build_program(b_loc=B_LOC):
    hw = HW
    m_loc = b_loc * hw
    n_chunk = m_loc // 128
    assert n_chunk * 128 == m_loc
    m_tot = N_CORES * m_loc
    nc = bacc.Bacc(
        "TRN2",
        target_bir_lowering=False,
        debug=False,
        enable_asserts=False,
        num_devices=N_CORES,
    )

    x_dram = nc.dram_tensor("x", [b_loc, C, hw], FP32, kind="ExternalInput")
    rot_dram = nc.dram_tensor("rot", [C, C], FP32, kind="ExternalInput")
    out_dram = nc.dram_tensor("out", [b_loc, C, hw], FP32, kind="ExternalOutput")

    with tile.TileContext(nc) as tc, ExitStack() as stack:
        consts = stack.enter_context(tc.tile_pool(name="consts", bufs=1))
        persist = stack.enter_context(tc.tile_pool(name="persist", bufs=1))

        # Constants via inline (NEFF-embedded) tensors.
        eye_bf_dram = nc.inline_tensor(np.eye(C).astype(NP_BF16), name="c_eye_bf")
        eye15_bf_dram = nc.inline_tensor(
            (1.5 * np.eye(C)).astype(NP_BF16), name="c_eye15_bf"
        )
        eye_f_dram = nc.inline_tensor(np.eye(C, dtype=np.float32), name="c_eye_f")
        ones_bf_dram = nc.inline_tensor(np.ones((C, C)).astype(NP_BF16), name="c_ones_bf")
        # --- dummy collective: absorbs the one-time cc-stream init barrier ---
        dummy_sb = consts.tile([1, 8], FP32)
        nc.vector.memset(dummy_sb, 0.0)
        with tc.tile_pool(name="dram_dummy", bufs=1, space="DRAM") as dummy_pool:
            dummy_in = dummy_pool.tile([1, 8], FP32)
            dummy_out = dummy_pool.tile([8, 8], FP32, addr_space="Shared")
            nc.sync.dma_start(dummy_in, dummy_sb)
            nc.gpsimd.collective_compute(
                "AllGather",
                ALU.bypass,
                replica_groups=[list(range(N_CORES))],
                ins=[dummy_in.opt()],
                outs=[dummy_out.opt()],
            )

        # --- load x and rot ---
        # batch 0 is loaded first, in halves, so the cast->transpose pipeline
        # starts ~4us earlier than with consts at the queue head.
        xsb = persist.tile([C, b_loc, hw], FP32)
        eye_bf = consts.tile([C, C], BF16)
        nc.sync.dma_start(eye_bf, eye_bf_dram[:])
        nc.sync.dma_start(out=xsb[:, 0, 0 : hw // 2], in_=x_dram[0, :, 0 : hw // 2])
        nc.sync.dma_start(out=xsb[:, 0, hw // 2 : hw], in_=x_dram[0, :, hw // 2 : hw])
        rot_sb = persist.tile([C, C], FP32)
        nc.sync.dma_start(out=rot_sb, in_=rot_dram[:])
        eye15_bf = consts.tile([C, C], BF16)
        nc.sync.dma_start(eye15_bf, eye15_bf_dram[:])
        eye_f = consts.tile([C, C], FP32)
        nc.sync.dma_start(eye_f, eye_f_dram[:])
        ones_bf = consts.tile([C, C], BF16)
        nc.sync.dma_start(ones_bf, ones_bf_dram[:])
        # half-batch loads: each cast unlocks on its half's completion receipt
        for b in range(1, b_loc):
            nc.sync.dma_start(
                out=xsb[:, b, 0 : hw // 2], in_=x_dram[b, :, 0 : hw // 2]
            )
            nc.sync.dma_start(
                out=xsb[:, b, hw // 2 : hw], in_=x_dram[b, :, hw // 2 : hw]
            )
        xflat = xsb.rearrange("p a b -> p (a b)")

        # bf16 copy of x, cast in half-batch chunks alternating DVE/ACT.
        xbf = persist.tile([C, m_loc], BF16)
        half = hw // 2  # 1568
        for k in range(2 * b_loc):
            dst = xbf[:, k * half : (k + 1) * half]
            src = xflat[:, k * half : (k + 1) * half]
            if k % 2 == 0:
                nc.vector.tensor_copy(dst, src)
            else:
                nc.scalar.copy(dst, src)

        # rot^T in bf16 (independent of stats; runs during phase 1)
        rot_bf = persist.tile([C, C], BF16)
        nc.vector.tensor_copy(rot_bf, rot_sb)

        # --- phase 1: Gram + channel sums, all bf16 on the PE ---
        # Transposed chunks are evicted PSUM->SBUF four-at-a-time (one strided
        # copy per 4 chunks): phase 1 was eviction-paced at ~290ns/chunk with
        # per-chunk copies; quad eviction brings it to ~100ns/chunk so the PE
        # (MMs issue ~56-81ns apart warm, ldweights hidden by the background
        # weight buffer) sets the pace instead.
        N_QSTRIP = 6
        qstrips = [
            persist.tile([C, 4, C + 1], BF16, name=f"qstrip{i}") for i in range(N_QSTRIP)
        ]
        for qs in qstrips:
            nc.vector.memset(qs[:, :, C : C + 1], 1.0)

        rotT_bf = persist.tile([C, C], BF16)

        with (
            tc.tile_pool(name="ph1_psum", bufs=6, space=bass.MemorySpace.PSUM) as ph1_psum,
            tc.tile_pool(name="gs_psum_pool", bufs=1, space=bass.MemorySpace.PSUM) as gs_pool,
        ):
            gs_psum = gs_pool.tile([C, C + 1], FP32)
            n_quad = n_chunk // 4  # 49
            for q in range(n_quad):
                y_ps = ph1_psum.tile([C, 4 * C], BF16, tag="ytrans")
                ypsv = y_ps.rearrange("p (s c) -> p s c", s=4)
                for s in range(4):
                    j = 4 * q + s
                    nc.tensor.transpose(
                        ypsv[:, s, :], xbf[:, j * 128 : (j + 1) * 128], eye_bf
                    )
                qs = qstrips[q % N_QSTRIP]
                if q % 2 == 0:
                    nc.vector.tensor_copy(qs[:, :, 0:C], ypsv)
                else:
                    nc.scalar.copy(qs[:, :, 0:C], ypsv)
                for s in range(4):
                    j = 4 * q + s
                    nc.tensor.matmul(
                        gs_psum,
                        qs[:, s, 0:C],
                        qs[:, s, 0 : C + 1],
                        start=(j == 0),
                        stop=(j == n_chunk - 1),
                    )

            gs_sb = persist.tile([C, C + 1], FP32)
            nc.vector.tensor_copy(gs_sb, gs_psum)

            # rot transpose (PE, once) — emitted AFTER the chunk loop so it
            # doesn't gate the first chunk transpose on the rot DMA; only
            # needed at mt, ~6us after the AllReduce returns.
            rotT_ps = gs_pool.tile([C, C], BF16, tag="rott")
            nc.tensor.transpose(rotT_ps, rot_bf, eye_bf)
            nc.scalar.copy(rotT_bf, rotT_ps)

        # --- AllReduce [G|s] across the 8 cores ---
        gtot = persist.tile([C, C + 1], FP32)
        with tc.tile_pool(name="dram", bufs=1, space="DRAM") as dram_pool:
            cc_in = dram_pool.tile([C, C + 1], FP32)
            cc_out = dram_pool.tile([C, C + 1], FP32, addr_space="Shared")
            nc.sync.dma_start(cc_in, gs_sb)
            nc.gpsimd.collective_compute(
                "AllReduce",
                ALU.add,
                replica_groups=[list(range(N_CORES))],
                ins=[cc_in.opt()],
                outs=[cc_out.opt()],
            )
            nc.sync.dma_start(gtot, cc_out)

        # --- phase 2: small replicated math ---
        with tc.tile_pool(name="ph2_psum", bufs=4, space=bass.MemorySpace.PSUM) as pp:
            inv_m = float(1.0 / m_tot)
            mean_bf = persist.tile([C, 1], BF16)
            nc.vector.tensor_scalar_mul(mean_bf, gtot[:, C : C + 1], inv_m)

            # trace: diag extract (single nonzero per row -> exact even in bf16)
            dmul_bf = persist.tile([C, C], BF16)
            nc.vector.tensor_mul(dmul_bf, gtot[:, 0:C], eye_f)
            diag_bf = persist.tile([C, 1], BF16)
            with nc.allow_low_precision(reason="single nonzero per row; exact"):
                nc.vector.tensor_reduce(diag_bf, dmul_bf, AX.X, ALU.add)
            trace_ps = pp.tile([C, 1], FP32, tag="ph2")
            nc.tensor.matmul(trace_ps, ones_bf, diag_bf, start=True, stop=True)
            # rtr = 1 / (trace(G)/m + C*eps)
            tr_sc = persist.tile([C, 1], FP32)
            nc.vector.tensor_scalar(
                tr_sc, trace_ps, inv_m, float(C * EPS), ALU.mult, ALU.add
            )
            rtr = persist.tile([C, 1], FP32)
            nc.vector.reciprocal(rtr, tr_sc)

            # sigN = (G/m) * rtr in bf16 (eps*rtr ~6e-8 vs diag ~8e-3: dropped)
            rtr_m = persist.tile([C, 1], FP32)
            nc.vector.tensor_scalar_mul(rtr_m, rtr, inv_m)
            sigN_bf = persist.tile([C, C], BF16)
            nc.vector.tensor_scalar_mul(sigN_bf, gtot[:, 0:C], rtr_m)

            # Newton in bf16: P <- 1.5 P - 0.5 P^3 SigmaN, P1 analytic.
            # P1 = 1.5 I - 0.5 SigmaN  (one DVE op + add of 1.5I const)
            p1_bf = persist.tile([C, C], BF16)
            nh_bf = persist.tile([C, C], BF16)
            nc.vector.tensor_scalar_mul(nh_bf, sigN_bf, -0.5)
            nc.vector.tensor_add(p1_bf, nh_bf, eye15_bf)
            pcur = p1_bf
            # remaining iterations with the update fused into PSUM accumulation:
            #   a = -0.5 P^2 (scaled on evict; exact in bf16)
            #   c = a @ (P Sig) ; c += P @ (1.5 I)  -> c = 1.5P - 0.5 P^3 Sig
            ptiles = [persist.tile([C, C], BF16, name=f"pnewt{i}") for i in range(2)]
            ab_t = [persist.tile([C, C], BF16, name=f"abuf{i}") for i in range(2)]
            db_t = [persist.tile([C, C], BF16, name=f"dbuf{i}") for i in range(2)]
            for it in range(T_NEWTON - 1):
                a_bf, d_bf = ab_t[it % 2], db_t[it % 2]
                a_ps = pp.tile([C, C], FP32, tag="ph2")
                d_ps = pp.tile([C, C], FP32, tag="ph2")
                # d first: its eviction runs on the slower ACT engine
                nc.tensor.matmul(d_ps, pcur, sigN_bf, start=True, stop=True)  # P Sig
                nc.tensor.matmul(a_ps, pcur, pcur, start=True, stop=True)     # P^2
                nc.scalar.copy(d_bf, d_ps)
                nc.vector.tensor_scalar_mul(a_bf, a_ps, -0.5)
                c_ps = pp.tile([C, C], FP32, tag="ph2")
                nc.tensor.matmul(c_ps, a_bf, d_bf, start=True, stop=False)  # -0.5 P^3 S
                nc.tensor.matmul(c_ps, pcur, eye15_bf, start=False, stop=True)  # +1.5P
                pnext = ptiles[it % 2]
                nc.vector.tensor_copy(pnext, c_ps)
                pcur = pnext

            # srtr = sqrt(rtr) via 2 Newton steps on DVE, seed s0 = sqrt(1/128)
            # (emitted after the iteration loop so the DVE runs it during the
            # PE-bound stretches instead of in front of the critical chain)
            s0 = float(np.sqrt(1.0 / C))
            t_a = persist.tile([C, 1], FP32)
            nc.vector.tensor_scalar(
                t_a, rtr, 0.5 / s0, 0.5 * s0, ALU.mult, ALU.add
            )  # s1 = (rtr/s0 + s0)/2
            t_r = persist.tile([C, 1], FP32)
            nc.vector.reciprocal(t_r, t_a)                    # 1/s1
            t_b = persist.tile([C, 1], FP32)
            nc.vector.tensor_mul(t_b, rtr, t_r)               # rtr/s1
            srtr = persist.tile([C, 1], FP32)
            nc.vector.tensor_add(srtr, t_a, t_b)
            nc.vector.tensor_scalar_mul(srtr, srtr, 0.5)      # s2

            # MT = sqrt(rTr) * P rot^T = M^T  (P symmetric)
            mt_ps = pp.tile([C, C], FP32, tag="ph2")
            nc.tensor.matmul(mt_ps, pcur, rotT_bf, start=True, stop=True)
            mt_bf = persist.tile([C, C], BF16)
            nc.vector.tensor_scalar_mul(mt_bf, mt_ps, srtr)

            # negbias = -(M @ mean)
            nb_ps = pp.tile([C, 1], FP32, tag="ph2")
            nc.tensor.matmul(nb_ps, mt_bf, mean_bf, start=True, stop=True)
            nb_sb = persist.tile([C, 1], FP32)
            nc.vector.tensor_scalar_mul(nb_sb, nb_ps, -1.0)

        # --- phase 3: out = M @ x - bias ---
        n_full, rem = divmod(hw, 512)  # 6, 64
        widths = [512] * n_full + ([rem] if rem else [])
        with (
            tc.tile_pool(name="ph3_psum", bufs=4, space=bass.MemorySpace.PSUM) as op_ps,
            tc.tile_pool(name="outsb_pool", bufs=2) as outsb_pool,
        ):
            for b in range(b_loc):
                osb = outsb_pool.tile([C, hw], FP32)
                col = 0
                for k, wdt in enumerate(widths):
                    ops = op_ps.tile([C, 512], FP32, tag="ops")
                    nc.tensor.matmul(
                        ops[:, 0:wdt],
                        mt_bf,
                        xbf[:, b * hw + col : b * hw + col + wdt],
                        start=True,
                        stop=True,
                    )
                    if k % 2 == 0:
                        nc.vector.tensor_scalar_add(
                            osb[:, col : col + wdt], ops[:, 0:wdt], nb_sb
                        )
                    else:
                        nc.scalar.add(
                            osb[:, col : col + wdt], ops[:, 0:wdt], nb_sb[:, 0:1]
                        )
                    col += wdt
                # split the writeback so streaming starts mid-batch
                nc.sync.dma_start(out=out_dram[b, :, 0:1536], in_=osb[:, 0:1536])
                nc.sync.dma_start(out=out_dram[b, :, 1536:hw], in_=osb[:, 1536:hw])

    nc.compile()
    return nc


_PROGRAM = None


def _get_program():
    global _PROGRAM
    if _PROGRAM is None:
        _PROGRAM = _build_program()
    return _PROGRAM


LAST_RESULTS = None


def kernel(x: np.ndarray, running_rot: np.ndarray) -> np.ndarray:
    global LAST_RESULTS
    x = np.ascontiguousarray(np.asarray(x, dtype=np.float32))
    rot = np.ascontiguousarray(np.asarray(running_rot, dtype=np.float32))
    assert x.shape == (B, C, H, W) and rot.shape == (C, C)

    nc = _get_program()
    xr = x.reshape(N_CORES, B_LOC, C, HW)
    in_maps = [{"x": xr[i], "rot": rot} for i in range(N_CORES)]
    res = bass_utils.run_bass_kernel_spmd(nc, in_maps, list(range(N_CORES)))
    LAST_RESULTS = res

    out = np.empty((B, C, H, W), dtype=np.float32)
    for i in range(N_CORES):
        out[i * B_LOC : (i + 1) * B_LOC] = res.results[i]["out"].reshape(
            B_LOC, C, H, W
        )
    return out


# revision 21
# speedup vs baseline: 1.0895x; 1.0784x over previous
"""Concept Whitening layer (IterNorm ZCA + rotation) as a Trainium2 Bass/Tile kernel.

Strategy (8-way data parallel over batch), v3 — bf16 compute, barrier-absorbed
AllReduce, fused Newton:
  - Each core holds 8 of the 64 batches.  x arrives fp32 (12.8 MB/core) and is
    cast to a bf16 SBUF-resident copy (DVE/ACT alternating, half-batch chunks).
    Output tolerance is 2e-2; full-bf16 numerics measure ~3e-3 end to end.
  - A 32-byte dummy AllReduce is issued first: the one-time collective-stream
    init barrier (measured 40-119us, gated on cross-core launch skew) completes
    during phase 1 instead of serializing in front of the real collective.
  - Phase 1: per 128-column chunk, PE-transpose (bf16) the chunk, evict to a
    rotating SBUF strip with a ones-column appended, accumulate
    [G | s] = y^T [y | 1] into one fp32 PSUM tile (196 accumulating matmuls).
  - AllReduce the (128,129) [G|s] (AG and AR both measure ~26us here; AR saves
    the 8 gather DMAs + 7-add local reduce tree).
  - Phase 2 (replicated): trace via diag mask + ones-matmul broadcast,
    rtr = 1/tr(Sigma), Newton for Sigma^{-1/2} in bf16 with P1 = 1.5I - 0.5*SigN
    computed analytically (saves one iteration) and the update fused into PSUM:
    evict P^2 scaled by -0.5 (exact in bf16), c = (-0.5 P^2)(P Sig), then
    c += P @ (1.5 I) accumulated in PSUM -> single evict per iteration.
  - Phase 3: out = M @ x - bias as 512-col bf16 matmuls over the resident
    x_bf16, PSUM->SBUF eviction with fused bias add alternating DVE/ACT,
    streamed to HBM per batch (double buffered).

out[b,d,h,w] = sum_c rot[d,c] * (wm @ (x-mean))[c] == (rot@wm) @ x - (rot@wm) @ mean.
"""

import sys

try:
    import concourse  # noqa: F401  (normally on PYTHONPATH in this container)
except ImportError:
    sys.path.insert(0, "/opt/trn_rl_repo")

from contextlib import ExitStack

import numpy as np

import concourse.bacc as bacc
import concourse.bass as bass
import concourse.mybir as mybir
import concourse.tile as tile
from concourse import bass_utils

# Problem constants (hardcoded per harness contract).
B, C, H, W = 64, 128, 56, 56
HW = H * W                    # 3136
M_TOT = B * HW                # 200704
N_CORES = 8
B_LOC = B // N_CORES          # 8
M_LOC = B_LOC * HW            # 25088
N_CHUNK = M_LOC // 128        # 196
T_NEWTON = 5
EPS = 1e-5

FP32 = mybir.dt.float32
BF16 = mybir.dt.bfloat16
AX = mybir.AxisListType
ALU = mybir.AluOpType
ACTF = mybir.ActivationFunctionType

NP_BF16 = mybir.dt.np(BF16)


def _build_program(b_loc=B_LOC):
    hw = HW
    m_loc = b_loc * hw
    n_chunk = m_loc // 128
    assert n_chunk * 128 == m_loc
    m_tot = N_CORES * m_loc
    nc = bacc.Bacc(
        "TRN2",
        target_bir_lowering=False,
        debug=False,
        enable_asserts=False,
        num_devices=N_CORES,
    )

    x_dram = nc.dram_tensor("x", [b_loc, C, hw], FP32, kind="ExternalInput")
    rot_dram = nc.dram_tensor("rot", [C, C], FP32, kind="ExternalInput")
    out_dram = nc.dram_tensor("out", [b_loc, C, hw], FP32, kind="ExternalOutput")

    with tile.TileContext(nc) as tc, ExitStack() as stack:
        consts = stack.enter_context(tc.tile_pool(name="consts", bufs=1))
        persist = stack.enter_context(tc.tile_pool(name="persist", bufs=1))

        # Constants via inline (NEFF-embedded) tensors.
        eye_bf_dram = nc.inline_tensor(np.eye(C).astype(NP_BF16), name="c_eye_bf")
        eye15_bf_dram = nc.inline_tensor(
            (1.5 * np.eye(C)).astype(NP_BF16), name="c_eye15_bf"
        )
        eye_f_dram = nc.inline_tensor(np.eye(C, dtype=np.float32), name="c_eye_f")
        ones_bf_dram = nc.inline_tensor(np.ones((C, C)).astype(NP_BF16), name="c_ones_bf")
        # --- dummy collective: absorbs the one-time cc-stream init barrier ---
        dummy_sb = consts.tile([1, 8], FP32)
        nc.vector.memset(dummy_sb, 0.0)
        with tc.tile_pool(name="dram_dummy", bufs=1, space="DRAM") as dummy_pool:
            dummy_in = dummy_pool.tile([1, 8], FP32)
            dummy_out = dummy_pool.tile([8, 8], FP32, addr_space="Shared")
            nc.sync.dma_start(dummy_in, dummy_sb)
            nc.gpsimd.collective_compute(
                "AllGather",
                ALU.bypass,
                replica_groups=[list(range(N_CORES))],
                ins=[dummy_in.opt()],
                outs=[dummy_out.opt()],
            )

        # --- load x and rot ---
        # batch 0 is loaded first, in halves, so the cast->transpose pipeline
        # starts ~4us earlier than with consts at the queue head.
        xsb = persist.tile([C, b_loc, hw], FP32)
        eye_bf = consts.tile([C, C], BF16)
        nc.sync.dma_start(eye_bf, eye_bf_dram[:])
        nc.sync.dma_start(out=xsb[:, 0, 0 : hw // 2], in_=x_dram[0, :, 0 : hw // 2])
        nc.sync.dma_start(out=xsb[:, 0, hw // 2 : hw], in_=x_dram[0, :, hw // 2 : hw])
        rot_sb = persist.tile([C, C], FP32)
        nc.sync.dma_start(out=rot_sb, in_=rot_dram[:])
        eye15_bf = consts.tile([C, C], BF16)
        nc.sync.dma_start(eye15_bf, eye15_bf_dram[:])
        eye_f = consts.tile([C, C], FP32)
        nc.sync.dma_start(eye_f, eye_f_dram[:])
        ones_bf = consts.tile([C, C], BF16)
        nc.sync.dma_start(ones_bf, ones_bf_dram[:])
        # half-batch loads: each cast unlocks on its half's completion receipt
        for b in range(1, b_loc):
            nc.sync.dma_start(
                out=xsb[:, b, 0 : hw // 2], in_=x_dram[b, :, 0 : hw // 2]
            )
            nc.sync.dma_start(
                out=xsb[:, b, hw // 2 : hw], in_=x_dram[b, :, hw // 2 : hw]
            )
        xflat = xsb.rearrange("p a b -> p (a b)")

        # bf16 copy of x, cast in half-batch chunks alternating DVE/ACT.
        xbf = persist.tile([C, m_loc], BF16)
        half = hw // 2  # 1568
        for k in range(2 * b_loc):
            dst = xbf[:, k * half : (k + 1) * half]
            src = xflat[:, k * half : (k + 1) * half]
            if k % 2 == 0:
                nc.vector.tensor_copy(dst, src)
            else:
                nc.scalar.copy(dst, src)

        # rot^T in bf16 (independent of stats; runs during phase 1)
        rot_bf = persist.tile([C, C], BF16)
        nc.vector.tensor_copy(rot_bf, rot_sb)

        # --- phase 1: Gram + channel sums, all bf16 on the PE ---
        # Transposed chunks are evicted PSUM->SBUF four-at-a-time (one strided
        # copy per 4 chunks): phase 1 was eviction-paced at ~290ns/chunk with
        # per-chunk copies; quad eviction brings it to ~100ns/chunk so the PE
        # (MMs issue ~56-81ns apart warm, ldweights hidden by the background
        # weight buffer) sets the pace instead.
        N_QSTRIP = 6
        qstrips = [
            persist.tile([C, 4, C + 1], BF16, name=f"qstrip{i}") for i in range(N_QSTRIP)
        ]
        for qs in qstrips:
            nc.vector.memset(qs[:, :, C : C + 1], 1.0)

        rotT_bf = persist.tile([C, C], BF16)

        with (
            tc.tile_pool(name="ph1_psum", bufs=6, space=bass.MemorySpace.PSUM) as ph1_psum,
            tc.tile_pool(name="gs_psum_pool", bufs=1, space=bass.MemorySpace.PSUM) as gs_pool,
        ):
            gs_psum = gs_pool.tile([C, C + 1], FP32)
            n_quad = n_chunk // 4  # 49
            for q in range(n_quad):
                y_ps = ph1_psum.tile([C, 4 * C], BF16, tag="ytrans")
                ypsv = y_ps.rearrange("p (s c) -> p s c", s=4)
                for s in range(4):
                    j = 4 * q + s
                    nc.tensor.transpose(
                        ypsv[:, s, :], xbf[:, j * 128 : (j + 1) * 128], eye_bf
                    )
                qs = qstrips[q % N_QSTRIP]
                if q % 2 == 0:
                    nc.vector.tensor_copy(qs[:, :, 0:C], ypsv)
                else:
                    nc.scalar.copy(qs[:, :, 0:C], ypsv)
                for s in range(4):
                    j = 4 * q + s
                    nc.tensor.matmul(
                        gs_psum,
                        qs[:, s, 0:C],
                        qs[:, s, 0 : C + 1],
                        start=(j == 0),
                        stop=(j == n_chunk - 1),
                    )

            gs_sb = persist.tile([C, C + 1], FP32)
            nc.vector.tensor_copy(gs_sb, gs_psum)

            # rot transpose (PE, once) — emitted AFTER the chunk loop so it
            # doesn't gate the first chunk transpose on the rot DMA; only
            # needed at mt, ~6us after the AllReduce returns.
            rotT_ps = gs_pool.tile([C, C], BF16, tag="rott")
            nc.tensor.transpose(rotT_ps, rot_bf, eye_bf)
            nc.scalar.copy(rotT_bf, rotT_ps)

        # --- AllReduce [G|s] across the 8 cores ---
        gtot = persist.tile([C, C + 1], FP32)
        with tc.tile_pool(name="dram", bufs=1, space="DRAM") as dram_pool:
            cc_in = dram_pool.tile([C, C + 1], FP32)
            cc_out = dram_pool.tile([C, C + 1], FP32, addr_space="Shared")
            nc.sync.dma_start(cc_in, gs_sb)
            nc.gpsimd.collective_compute(
                "AllReduce",
                ALU.add,
                replica_groups=[list(range(N_CORES))],
                ins=[cc_in.opt()],
                outs=[cc_out.opt()],
            )
            nc.sync.dma_start(gtot, cc_out)

        # --- phase 2: small replicated math ---
        with tc.tile_pool(name="ph2_psum", bufs=4, space=bass.MemorySpace.PSUM) as pp:
            inv_m = float(1.0 / m_tot)
            mean_bf = persist.tile([C, 1], BF16)
            nc.vector.tensor_scalar_mul(mean_bf, gtot[:, C : C + 1], inv_m)

            # trace: diag extract (single nonzero per row -> exact even in bf16)
            dmul_bf = persist.tile([C, C], BF16)
            nc.vector.tensor_mul(dmul_bf, gtot[:, 0:C], eye_f)
            diag_bf = persist.tile([C, 1], BF16)
            with nc.allow_low_precision(reason="single nonzero per row; exact"):
                nc.vector.tensor_reduce(diag_bf, dmul_bf, AX.X, ALU.add)
            trace_ps = pp.tile([C, 1], FP32, tag="ph2")
            nc.tensor.matmul(trace_ps, ones_bf, diag_bf, start=True, stop=True)
            # rtr = 1 / (trace(G)/m + C*eps)
            tr_sc = persist.tile([C, 1], FP32)
            nc.vector.tensor_scalar(
                tr_sc, trace_ps, inv_m, float(C * EPS), ALU.mult, ALU.add
            )
            rtr = persist.tile([C, 1], FP32)
            nc.vector.reciprocal(rtr, tr_sc)

            # sigN = (G/m) * rtr in bf16 (eps*rtr ~6e-8 vs diag ~8e-3: dropped)
            rtr_m = persist.tile([C, 1], FP32)
            nc.vector.tensor_scalar_mul(rtr_m, rtr, inv_m)
            sigN_bf = persist.tile([C, C], BF16)
            nc.vector.tensor_scalar_mul(sigN_bf, gtot[:, 0:C], rtr_m)

            # Newton in bf16: P <- 1.5 P - 0.5 P^3 SigmaN, P1 analytic.
            # P1 = 1.5 I - 0.5 SigmaN  (one DVE op + add of 1.5I const)
            p1_bf = persist.tile([C, C], BF16)
            nh_bf = persist.tile([C, C], BF16)
            nc.vector.tensor_scalar_mul(nh_bf, sigN_bf, -0.5)
            nc.vector.tensor_add(p1_bf, nh_bf, eye15_bf)
            pcur = p1_bf
            # remaining iterations with the update fused into PSUM accumulation:
            #   a = -0.5 P^2 (scaled on evict; exact in bf16)
            #   c = a @ (P Sig) ; c += P @ (1.5 I)  -> c = 1.5P - 0.5 P^3 Sig
            ptiles = [persist.tile([C, C], BF16, name=f"pnewt{i}") for i in range(2)]
            ab_t = [persist.tile([C, C], BF16, name=f"abuf{i}") for i in range(2)]
            db_t = [persist.tile([C, C], BF16, name=f"dbuf{i}") for i in range(2)]
            for it in range(T_NEWTON - 1):
                a_bf, d_bf = ab_t[it % 2], db_t[it % 2]
                a_ps = pp.tile([C, C], FP32, tag="ph2")
                d_ps = pp.tile([C, C], FP32, tag="ph2")
                # d first: its eviction runs on the slower ACT engine
                nc.tensor.matmul(d_ps, pcur, sigN_bf, start=True, stop=True)  # P Sig
                nc.tensor.matmul(a_ps, pcur, pcur, start=True, stop=True)     # P^2
                c_ps = pp.tile([C, C], FP32, tag="ph2")
                # 1.5P lands in PSUM while the evictions below run (depends
                # only on pcur), so the post-evict segment is just a@d.
                nc.tensor.matmul(c_ps, pcur, eye15_bf, start=True, stop=False)  # 1.5P
                nc.scalar.copy(d_bf, d_ps)
                nc.vector.tensor_scalar_mul(a_bf, a_ps, -0.5)
                nc.tensor.matmul(c_ps, a_bf, d_bf, start=False, stop=True)  # -0.5 P^3 S
                pnext = ptiles[it % 2]
                nc.vector.tensor_copy(pnext, c_ps)
                pcur = pnext

            # srtr = sqrt(rtr) via 2 Newton steps on DVE, seed s0 = sqrt(1/128)
            # (emitted after the iteration loop so the DVE runs it during the
            # PE-bound stretches instead of in front of the critical chain)
            s0 = float(np.sqrt(1.0 / C))
            t_a = persist.tile([C, 1], FP32)
            nc.vector.tensor_scalar(
                t_a, rtr, 0.5 / s0, 0.5 * s0, ALU.mult, ALU.add
            )  # s1 = (rtr/s0 + s0)/2
            t_r = persist.tile([C, 1], FP32)
            nc.vector.reciprocal(t_r, t_a)                    # 1/s1
            t_b = persist.tile([C, 1], FP32)
            nc.vector.tensor_mul(t_b, rtr, t_r)               # rtr/s1
            srtr = persist.tile([C, 1], FP32)
            nc.vector.tensor_add(srtr, t_a, t_b)
            nc.vector.tensor_scalar_mul(srtr, srtr, 0.5)      # s2

            # MT = sqrt(rTr) * P rot^T = M^T  (P symmetric)
            mt_ps = pp.tile([C, C], FP32, tag="ph2")
            nc.tensor.matmul(mt_ps, pcur, rotT_bf, start=True, stop=True)
            mt_bf = persist.tile([C, C], BF16)
            nc.vector.tensor_scalar_mul(mt_bf, mt_ps, srtr)

            # negbias = -(M @ mean)
            nb_ps = pp.tile([C, 1], FP32, tag="ph2")
            nc.tensor.matmul(nb_ps, mt_bf, mean_bf, start=True, stop=True)
            nb_sb = persist.tile([C, 1], FP32)
            nc.vector.tensor_scalar_mul(nb_sb, nb_ps, -1.0)

        # --- phase 3: out = M @ x - bias ---
        n_full, rem = divmod(hw, 512)  # 6, 64
        widths = [512] * n_full + ([rem] if rem else [])
        with (
            tc.tile_pool(name="ph3_psum", bufs=4, space=bass.MemorySpace.PSUM) as op_ps,
            tc.tile_pool(name="outsb_pool", bufs=2) as outsb_pool,
        ):
            for b in range(b_loc):
                osb = outsb_pool.tile([C, hw], FP32)
                col = 0
                for k, wdt in enumerate(widths):
                    ops = op_ps.tile([C, 512], FP32, tag="ops")
                    nc.tensor.matmul(
                        ops[:, 0:wdt],
                        mt_bf,
                        xbf[:, b * hw + col : b * hw + col + wdt],
                        start=True,
                        stop=True,
                    )
                    if k % 2 == 0:
                        nc.vector.tensor_scalar_add(
                            osb[:, col : col + wdt], ops[:, 0:wdt], nb_sb
                        )
                    else:
                        nc.scalar.add(
                            osb[:, col : col + wdt], ops[:, 0:wdt], nb_sb[:, 0:1]
                        )
                    col += wdt
                # split the writeback so streaming starts mid-batch; the last
                # batch drains in quarters to shorten the final tail
                if b < b_loc - 1:
                    nc.sync.dma_start(out=out_dram[b, :, 0:1536], in_=osb[:, 0:1536])
                    nc.sync.dma_start(out=out_dram[b, :, 1536:hw], in_=osb[:, 1536:hw])
                else:
                    for c0, c1 in ((0, 1024), (1024, 2048), (2048, 2560), (2560, hw)):
                        nc.sync.dma_start(
                            out=out_dram[b, :, c0:c1], in_=osb[:, c0:c1]
                        )

    nc.compile()
    return nc


_PROGRAM = None


def _get_program():
    global _PROGRAM
    if _PROGRAM is None:
        _PROGRAM = _build_program()
    return _PROGRAM


LAST_RESULTS = None


def kernel(x: np.ndarray, running_rot: np.ndarray) -> np.ndarray:
    global LAST_RESULTS
    x = np.ascontiguousarray(np.asarray(x, dtype=np.float32))
    rot = np.ascontiguousarray(np.asarray(running_rot, dtype=np.float32))
    assert x.shape == (B, C, H, W) and rot.shape == (C, C)

    nc = _get_program()
    xr = x.reshape(N_CORES, B_LOC, C, HW)
    in_maps = [{"x": xr[i], "rot": rot} for i in range(N_CORES)]
    res = bass_utils.run_bass_kernel_spmd(nc, in_maps, list(range(N_CORES)))
    LAST_RESULTS = res

    out = np.empty((B, C, H, W), dtype=np.float32)
    for i in range(N_CORES):
        out[i * B_LOC : (i + 1) * B_LOC] = res.results[i]["out"].reshape(
            B_LOC, C, H, W
        )
    return out
